# revision 17
# baseline (speedup 1.0000x reference)
"""Trainium2 Bass kernel for a 2-layer GraphConv GCN (nn_GCNN_69776038691375).

reference semantics:
    x = h.swapaxes(0,1)                       # [N, B, F]
    out_deg/in_deg from src/dst, clipped at 1
    s = out_deg**-0.5 ; d = in_deg**-0.5
    layer(x, W, b) = (segsum((x*s)[src] -> dst) * d) @ W + b
    y = relu(layer(x, W1, b1)); out = layer(y, W2, b2); return out.swapaxes(0,1)

Key identity used on device: aggregation commutes with the feature transform,
so each layer computes  agg((x*s) @ W) * d + b  — for layer 2 this shrinks the
gathered rows from 256 to 128 floats.

Distribution (8 cores): destination-node sharding. Nodes padded to
NPAD=50176 = 8 cores x 49 blocks x 128 nodes. Core c owns global blocks
[c*49, (c+1)*49). Edges are grouped by dst block; within a block they are
split into lo (src < 25088) / hi halves because dma_gather indices are int16.
Aggregation = gathered rows (dma_gather) reduced with a one-hot matrix built
on device (is_equal against a column-index matrix) via TensorE matmuls
accumulating in PSUM. In-degree falls out of the same matmuls against a ones
column; out-degree comes from an identical counting pass over src-sorted
edges. s_norm (tiny) and the layer-2 table (25.7MB) are AllGathered on-chip.
"""

import numpy as np

import concourse.bacc as bacc
import concourse.bass as bass
import concourse.mybir as mybir
import concourse.tile as tile
from concourse.bass_interp import get_hw_module
from concourse.bass_utils import run_bass_kernel_spmd

F32 = mybir.dt.float32
I16 = mybir.dt.int16

# problem sizes (hardcoded per contract)
N = 50000
E = 800000
B = 4
IN_D, HID_D, OUT_D = 64, 64, 32
NCORES = 8
PB = 49                 # blocks per core
NB = NCORES * PB        # 392 global blocks
NPAD = NB * 128         # 50176
HALF = NPAD // 2        # 25088: dma_gather int16 index limit split point
D1 = B * HID_D          # 256 floats per layer-1 table row
D2 = B * OUT_D          # 128 floats per layer-2 table row
SENT = 250              # one-hot sentinel for padded edges
SPLIT = 24              # L1-loop block index after which the first y2w AllGather fires


# ---------------------------------------------------------------- host side

def _wrap_idx(flat):
    """dma_gather index layout: idx j of a gather lives at [j%16, j//16],
    replicated across the 8 groups of 16 partitions. flat: [T, 128] int16
    (subtile-major). Returns [128, T*8]."""
    T = flat.shape[0]
    w = flat.reshape(T, 8, 16).transpose(2, 0, 1).reshape(16, T * 8)
    return np.tile(w, (8, 1)).astype(np.int16)


def _preprocess(src, dst):
    """Build per-core padded edge structures. Returns (percore, C_lo, C_hi, Sd)."""
    src = np.asarray(src).astype(np.int64)
    dst = np.asarray(dst).astype(np.int64)

    # ---- dst-sorted structure for the aggregation passes
    blk = dst >> 7
    hi = (src >= HALF).astype(np.int64)
    order = np.lexsort((src, hi, blk))
    s_src, s_dst, s_blk, s_hi = src[order], dst[order], blk[order], hi[order]
    # counts per (block, half)
    cnt = np.bincount(s_blk * 2 + s_hi, minlength=NB * 2).reshape(NB, 2)
    starts = np.concatenate([[0], np.cumsum(cnt.ravel())])[:-1].reshape(NB, 2)
    # per block-index subtile counts, max over cores (shared program shape)
    lo_sub = -(-cnt[:, 0] // 128).reshape(NCORES, PB)
    hi_sub = -(-cnt[:, 1] // 128).reshape(NCORES, PB)
    C_lo = np.maximum(lo_sub.max(axis=0), 1).astype(int)
    C_hi = hi_sub.max(axis=0).astype(int)

    # ---- src-sorted structure for the out-degree pass
    sblk = src >> 7
    order2 = np.argsort(sblk, kind="stable")
    d_src, d_sblk = src[order2], sblk[order2]
    dcnt = np.bincount(d_sblk, minlength=NB)
    dstarts = np.concatenate([[0], np.cumsum(dcnt)])[:-1]
    dsub = -(-dcnt // 128).reshape(NCORES, PB)
    Sd = np.maximum(dsub.max(axis=0), 1).astype(int)

    T_agg = int(C_lo.sum() + C_hi.sum())
    T_deg = int(Sd.sum())

    # ---- L2 structure: table is the concat of two AllGather outputs:
    # A = per-core blocks 0..SPLIT-1 (chunk SPLIT*128 rows/rank),
    # B = per-core blocks SPLIT..PB-1. Positions fit int16.
    src_c = src // (PB * 128)
    src_b = (src % (PB * 128)) >> 7
    src_p = src & 127
    in_b2 = (src_b >= SPLIT).astype(np.int64)
    pos = np.where(in_b2 == 0,
                   src_c * (SPLIT * 128) + src_b * 128 + src_p,
                   src_c * ((PB - SPLIT) * 128) + (src_b - SPLIT) * 128 + src_p)
    order3 = np.lexsort((src, in_b2, blk))
    t_pos, t_dst, t_blk, t_b2 = pos[order3], dst[order3], blk[order3], in_b2[order3]
    cnt2 = np.bincount(t_blk * 2 + t_b2, minlength=NB * 2).reshape(NB, 2)
    starts2 = np.concatenate([[0], np.cumsum(cnt2.ravel())])[:-1].reshape(NB, 2)
    a_sub = -(-cnt2[:, 0] // 128).reshape(NCORES, PB)
    b_sub = -(-cnt2[:, 1] // 128).reshape(NCORES, PB)
    C_a = np.maximum(a_sub.max(axis=0), 1).astype(int)
    C_b = b_sub.max(axis=0).astype(int)
    T_ag2 = int(C_a.sum() + C_b.sum())

    percore = []
    for c in range(NCORES):
        gsl = []  # gather indices, [T_agg, 128] int16 (relative to half)
        dsl = []  # dst-local,      [T_agg, 128] int16
        for b in range(PB):
            g = c * PB + b
            base = g * 128
            for h, C in ((0, C_lo[b]), (1, C_hi[b])):
                n = int(cnt[g, h])
                st = int(starts[g, h])
                gi = np.zeros(C * 128, np.int16)
                dl = np.full(C * 128, SENT, np.int16)
                gi[:n] = (s_src[st:st + n] - h * HALF).astype(np.int16)
                dl[:n] = (s_dst[st:st + n] - base).astype(np.int16)
                gsl.append(gi.reshape(C, 128))
                dsl.append(dl.reshape(C, 128))
        gs = np.concatenate(gsl, axis=0)
        ds = np.concatenate(dsl, axis=0)
        sl = []  # src-local for degree pass, [T_deg, 128] int16
        for b in range(PB):
            g = c * PB + b
            n = int(dcnt[g])
            st = int(dstarts[g])
            s = np.full(Sd[b] * 128, SENT, np.int16)
            s[:n] = (d_src[st:st + n] - g * 128).astype(np.int16)
            sl.append(s.reshape(Sd[b], 128))
        sv = np.concatenate(sl, axis=0)
        gsl2, dsl2 = [], []
        for b in range(PB):
            g = c * PB + b
            base = g * 128
            for h, C in ((0, C_a[b]), (1, C_b[b])):
                n = int(cnt2[g, h])
                st = int(starts2[g, h])
                gi = np.zeros(C * 128, np.int16)
                dl = np.full(C * 128, SENT, np.int16)
                gi[:n] = t_pos[st:st + n].astype(np.int16)
                dl[:n] = (t_dst[st:st + n] - base).astype(np.int16)
                gsl2.append(gi.reshape(C, 128))
                dsl2.append(dl.reshape(C, 128))
        gs2 = np.concatenate(gsl2, axis=0)
        ds2 = np.concatenate(dsl2, axis=0)
        percore.append({
            "gidx": _wrap_idx(gs),            # [128, T_agg*8]
            "dstl": np.ascontiguousarray(ds.T),  # [128, T_agg]
            "srcl": np.ascontiguousarray(sv.T),  # [128, T_deg]
            "gidx2": _wrap_idx(gs2),             # [128, T_ag2*8]
            "dstl2": np.ascontiguousarray(ds2.T),  # [128, T_ag2]
        })
    meta = dict(C_lo=C_lo.tolist(), C_hi=C_hi.tolist(), Sd=Sd.tolist(),
                C_a=C_a.tolist(), C_b=C_b.tolist(),
                T_agg=T_agg, T_deg=T_deg, T_ag2=T_ag2)
    return percore, meta


# -------------------------------------------------------------- bass program

def _build(meta, collectives=True, upto='l2'):
    C_lo, C_hi, Sd = meta["C_lo"], meta["C_hi"], meta["Sd"]
    C_a, C_b = meta["C_a"], meta["C_b"]
    T_agg, T_deg, T_ag2 = meta["T_agg"], meta["T_deg"], meta["T_ag2"]
    CMAX = max(max(C_lo[b] + C_hi[b] for b in range(PB)),
               max(C_a[b] + C_b[b] for b in range(PB)), max(Sd))
    nc = bacc.Bacc("TRN2", target_bir_lowering=False, debug=False,
                   num_devices=NCORES)

    hT = nc.dram_tensor("hT", [B, IN_D, NPAD], F32, kind="ExternalInput")
    w1 = nc.dram_tensor("w1", [IN_D, HID_D], F32, kind="ExternalInput")
    w2 = nc.dram_tensor("w2", [HID_D, OUT_D], F32, kind="ExternalInput")
    b1r = nc.dram_tensor("b1r", [128, D1], F32, kind="ExternalInput")
    b2r = nc.dram_tensor("b2r", [128, D2], F32, kind="ExternalInput")
    jrep = nc.dram_tensor("jrep", [128, CMAX * 128], F32, kind="ExternalInput")
    ident = nc.dram_tensor("ident", [128, 128], F32, kind="ExternalInput")
    gidx = nc.dram_tensor("gidx", [128, T_agg * 8], I16, kind="ExternalInput")
    dstl = nc.dram_tensor("dstl", [128, T_agg], I16, kind="ExternalInput")
    gidx2 = nc.dram_tensor("gidx2", [128, T_ag2 * 8], I16, kind="ExternalInput")
    dstl2 = nc.dram_tensor("dstl2", [128, T_ag2], I16, kind="ExternalInput")
    srcl = nc.dram_tensor("srcl", [128, T_deg], I16, kind="ExternalInput")

    out_loc = nc.dram_tensor("out_loc", [PB * 128, D2], F32, kind="ExternalOutput")

    xw1_lo = nc.dram_tensor("xw1_lo", [HALF, D1], F32)
    xw1_hi = nc.dram_tensor("xw1_hi", [HALF, D1], F32)
    y2w_loc_a = nc.dram_tensor("y2w_loc_a", [SPLIT * 128, D2], F32)
    y2w_loc_b = nc.dram_tensor("y2w_loc_b", [(PB - SPLIT) * 128, D2], F32)
    y2w_full_a = nc.dram_tensor("y2w_full_a", [NCORES * SPLIT * 128, D2], F32,
                                addr_space="Shared")
    y2w_full_b = nc.dram_tensor("y2w_full_b", [NCORES * (PB - SPLIT) * 128, D2], F32,
                                addr_space="Shared")
    snorm_loc = nc.dram_tensor("snorm_loc", [128, PB], F32)
    snorm_full = nc.dram_tensor("snorm_full", [NCORES * 128, PB], F32,
                                addr_space="Shared")

    rg = [list(range(NCORES))]

    with tile.TileContext(nc) as tc:
        with (
            tc.tile_pool(name="persist", bufs=1) as pp,
            tc.tile_pool(name="sbuf", bufs=2) as sb,
            tc.tile_pool(name="post", bufs=2) as pq,
            tc.tile_pool(name="psA", bufs=4, space="PSUM") as psA,
            tc.tile_pool(name="psB", bufs=2, space="PSUM") as psB,
            tc.tile_pool(name="psC", bufs=1, space="PSUM") as psC,
        ):
            # ---- constants / persistent state
            jr_t = pp.tile([128, CMAX * 128], F32)
            nc.sync.dma_start(out=jr_t[:], in_=jrep[:])
            id_t = pp.tile([128, 128], F32)
            nc.sync.dma_start(out=id_t[:], in_=ident[:])
            w1_t = pp.tile([IN_D, HID_D], F32)
            nc.sync.dma_start(out=w1_t[:], in_=w1[:])
            w2_t = pp.tile([HID_D, OUT_D], F32)
            nc.sync.dma_start(out=w2_t[:], in_=w2[:])
            b1_t = pp.tile([128, D1], F32)
            nc.sync.dma_start(out=b1_t[:], in_=b1r[:])
            b2_t = pp.tile([128, D2], F32)
            nc.sync.dma_start(out=b2_t[:], in_=b2r[:])
            ones_t = pp.tile([128, 1], F32)
            nc.vector.memset(ones_t[:], 1.0)
            gidx_t = pp.tile([128, T_agg * 8], I16)
            nc.sync.dma_start(out=gidx_t[:], in_=gidx[:])
            dstl_t = pp.tile([128, T_agg], I16)
            nc.sync.dma_start(out=dstl_t[:], in_=dstl[:])
            srcl_t = pp.tile([128, T_deg], I16)
            nc.sync.dma_start(out=srcl_t[:], in_=srcl[:])
            dstl_f = pp.tile([128, T_agg], F32)
            nc.vector.tensor_copy(dstl_f[:], dstl_t[:])
            gidx2_t = pp.tile([128, T_ag2 * 8], I16)
            nc.sync.dma_start(out=gidx2_t[:], in_=gidx2[:])
            dstl2_t = pp.tile([128, T_ag2], I16)
            nc.sync.dma_start(out=dstl2_t[:], in_=dstl2[:])
            dstl2_f = pp.tile([128, T_ag2], F32)
            nc.vector.tensor_copy(dstl2_f[:], dstl2_t[:])
            srcl_f = pp.tile([128, T_deg], F32)
            nc.vector.tensor_copy(srcl_f[:], srcl_t[:])
            s_loc = pp.tile([128, PB], F32)    # out-deg norm, own nodes
            d_loc = pp.tile([128, PB], F32)    # in-deg norm, own nodes
            s_all = pp.tile([128, NB], F32)    # out-deg norm, all nodes

            # ---- pass 1: out-degree -> s_loc
            off = 0
            for b in range(PB):
                S = Sd[b]
                deg_ps = psB.tile([128, 1], F32, space="PSUM", tag="deg")
                oh = sb.tile([128, CMAX * 128], F32, tag="ohb")
                nc.vector.tensor_tensor(
                    out=oh[:, :S * 128],
                    in0=srcl_f[:, off:off + S].to_broadcast([128, S, 128]),
                    in1=jr_t[:, :S * 128], op=mybir.AluOpType.is_equal)
                for s in range(S):
                    nc.tensor.matmul(deg_ps[:], lhsT=oh[:, s * 128:(s + 1) * 128],
                                     rhs=ones_t[:],
                                     start=(s == 0), stop=(s == S - 1))
                off += S
                t0 = pq.tile([128, 1], F32, tag="dtmp")
                nc.vector.tensor_scalar_max(t0[:], deg_ps[:], 1.0)
                t1 = pq.tile([128, 1], F32, tag="dtmp2")
                nc.scalar.activation(t1[:], t0[:], mybir.ActivationFunctionType.Sqrt)
                nc.vector.reciprocal(s_loc[:, b:b + 1], t1[:])
            nc.sync.dma_start(out=snorm_loc[:], in_=s_loc[:])
            if collectives:
                nc.gpsimd.collective_compute(
                    "AllGather", mybir.AluOpType.bypass, replica_groups=rg,
                    ins=[snorm_loc[:]], outs=[snorm_full[:]])
            else:
                for c in range(NCORES):
                    nc.sync.dma_start(out=snorm_full[c * 128:(c + 1) * 128, :],
                                      in_=snorm_loc[:])
            for c in range(NCORES):
                nc.sync.dma_start(out=s_all[:, c * PB:(c + 1) * PB],
                                  in_=snorm_full[c * 128:(c + 1) * 128, :])

            # ---- pass 2: xw1 = (x @ W1) * s  for ALL nodes (redundant per core)
            # loads batched over 8 blocks, stores over 4 (fewer DMA setups)
            GL, GS = 8, 4
            lhs = None
            t1_sb = None
            for g in range(NB if upto != 'deg' else 0):
                if g % GL == 0:
                    lhs = sb.tile([IN_D, B * GL * 128], F32, tag="t1lhs")
                    for bb in range(B):
                        nc.sync.dma_start(
                            out=lhs[:, bb * GL * 128:(bb + 1) * GL * 128],
                            in_=hT[bb, :, g * 128:(g + GL) * 128])
                if g % GS == 0:
                    t1_sb = sb.tile([128, GS * D1], F32, tag="t1sb")
                gg = g % GL
                t1_ps = psA.tile([128, D1], F32, space="PSUM", tag="bigps")
                for bb in range(B):
                    nc.tensor.matmul(
                        t1_ps[:, bb * HID_D:(bb + 1) * HID_D],
                        lhsT=lhs[:, bb * GL * 128 + gg * 128:bb * GL * 128 + (gg + 1) * 128],
                        rhs=w1_t[:], start=True, stop=True)
                nc.vector.tensor_scalar_mul(
                    t1_sb[:, (g % GS) * D1:(g % GS + 1) * D1], t1_ps[:],
                    s_all[:, g:g + 1])
                if g % GS == GS - 1:
                    g0 = g - (GS - 1)
                    tgt = xw1_lo if g0 < NB // 2 else xw1_hi
                    r0 = (g0 % (NB // 2)) * 128
                    nc.sync.dma_start(
                        out=tgt[r0:r0 + GS * 128, :].rearrange(
                            "(c p) f -> p c f", p=128),
                        in_=t1_sb[:])

            # ---- pass 3: layer-1 aggregation + layer-2 table build
            qctr = [0]

            def agg_block(b, off_sub, table_lo, table_hi, D,
                          Cls, Chs, gi_t, dl_f):
                """Emit gathers + one-hot matmuls for block b. Returns
                (agg_ps, deg_ps, n_sub)."""
                Cl, Ch = Cls[b], Chs[b]
                Ct = Cl + Ch
                g_t = sb.tile([128, Ct, D], F32, tag=f"gath{D}")
                for h, C, tab in ((0, Cl, table_lo), (1, Ch, table_hi)):
                    if C == 0:
                        continue
                    c0 = 0 if h == 0 else Cl
                    nc.gpsimd.dma_gather(
                        out_ap=g_t[:, c0:c0 + C, :], in_ap=tab[:],
                        idxs_ap=gi_t[:, (off_sub + c0) * 8:(off_sub + c0 + C) * 8],
                        num_idxs=C * 128, num_idxs_reg=C * 128,
                        elem_size=D, single_packet=False)
                agg_ps = psA.tile([128, D1], F32, space="PSUM", tag="bigps")
                if D == D1:
                    deg_ps = psB.tile([128, 1], F32, space="PSUM", tag="deg")
                else:
                    deg_ps = None
                oh = sb.tile([128, CMAX * 128], F32, tag="ohb")
                nc.vector.tensor_tensor(
                    out=oh[:, :Ct * 128],
                    in0=dl_f[:, off_sub:off_sub + Ct].to_broadcast([128, Ct, 128]),
                    in1=jr_t[:, :Ct * 128], op=mybir.AluOpType.is_equal)
                for cs in range(Ct):
                    ohc = oh[:, cs * 128:(cs + 1) * 128]
                    nc.tensor.matmul(agg_ps[:, :D], lhsT=ohc, rhs=g_t[:, cs, :],
                                     start=(cs == 0), stop=(cs == Ct - 1))
                    if D == D1:  # in-degree only needed once (layer 1)
                        nc.tensor.matmul(deg_ps[:], lhsT=ohc, rhs=ones_t[:],
                                         start=(cs == 0), stop=(cs == Ct - 1))
                return agg_ps, deg_ps, Ct

            off = 0
            for b in range(PB if upto not in ('deg', 't1') else 0):
                agg_ps, deg_ps, Ct = agg_block(b, off, xw1_lo, xw1_hi, D1,
                                               C_lo, C_hi, gidx_t, dstl_f)
                off += Ct
                # d_norm from in-degree
                t0 = pq.tile([128, 1], F32, tag="dtmp")
                nc.vector.tensor_scalar_max(t0[:], deg_ps[:], 1.0)
                t1 = pq.tile([128, 1], F32, tag="dtmp2")
                nc.scalar.activation(t1[:], t0[:], mybir.ActivationFunctionType.Sqrt)
                nc.vector.reciprocal(d_loc[:, b:b + 1], t1[:])
                # y1 = relu(agg * d + b1); y1s = y1 * s
                y1a = pq.tile([128, D1], F32, tag="y1a")
                nc.vector.tensor_scalar_mul(y1a[:], agg_ps[:], d_loc[:, b:b + 1])
                y1b = pq.tile([128, D1], F32, tag="y1b")
                nc.vector.tensor_tensor(out=y1b[:], in0=y1a[:], in1=b1_t[:],
                                        op=mybir.AluOpType.add)
                y1r = pq.tile([128, D1], F32, tag="y1r")
                nc.scalar.activation(y1r[:], y1b[:], mybir.ActivationFunctionType.Relu)
                y1s = pq.tile([128, D1], F32, tag="y1s")
                nc.vector.tensor_scalar_mul(y1s[:], y1r[:], s_loc[:, b:b + 1])
                # transform-2: y1w2 = y1s @ W2 (per batch), via PE transpose
                t2_ps = psC.tile([128, D2], F32, space="PSUM", tag="t2ps")
                for bb in range(B):
                    tr_ps = psC.tile([HID_D, 128], F32, space="PSUM", tag="trps")
                    nc.tensor.transpose(
                        tr_ps[:], y1s[:, bb * HID_D:(bb + 1) * HID_D], id_t[:])
                    tr_sb = pq.tile([HID_D, 128], F32, tag="trsb")
                    nc.vector.tensor_copy(tr_sb[:], tr_ps[:])
                    nc.tensor.matmul(
                        t2_ps[:, bb * OUT_D:(bb + 1) * OUT_D],
                        lhsT=tr_sb[:], rhs=w2_t[:], start=True, stop=True)
                t2_sb = pq.tile([128, D2], F32, tag="t2sb")
                nc.vector.tensor_copy(t2_sb[:], t2_ps[:])
                if b < SPLIT:
                    nc.sync.dma_start(out=y2w_loc_a[b * 128:(b + 1) * 128, :],
                                      in_=t2_sb[:])
                else:
                    nc.sync.dma_start(
                        out=y2w_loc_b[(b - SPLIT) * 128:(b - SPLIT + 1) * 128, :],
                        in_=t2_sb[:])
                if b == SPLIT - 1 and upto == 'l2':
                    # first table half exchanged while the rest of L1 runs
                    if collectives:
                        nc.gpsimd.collective_compute(
                            "AllGather", mybir.AluOpType.bypass, replica_groups=rg,
                            ins=[y2w_loc_a[:]], outs=[y2w_full_a[:]])
                    else:
                        for c in range(NCORES):
                            nc.sync.dma_start(
                                out=y2w_full_a[c * SPLIT * 128:(c + 1) * SPLIT * 128, :],
                                in_=y2w_loc_a[:])

            # ---- pass 4: exchange second table half
            if upto == 'l2':
                if collectives:
                    nc.gpsimd.collective_compute(
                        "AllGather", mybir.AluOpType.bypass, replica_groups=rg,
                        ins=[y2w_loc_b[:]], outs=[y2w_full_b[:]])
                else:
                    nb128 = (PB - SPLIT) * 128
                    for c in range(NCORES):
                        nc.sync.dma_start(
                            out=y2w_full_b[c * nb128:(c + 1) * nb128, :],
                            in_=y2w_loc_b[:])

            # ---- pass 5: layer-2 aggregation -> output
            off = 0
            for b in range(PB if upto == 'l2' else 0):
                agg_ps, _, Ct = agg_block(b, off, y2w_full_a, y2w_full_b, D2,
                                          C_a, C_b, gidx2_t, dstl2_f)
                off += Ct
                oa = pq.tile([128, D2], F32, tag="oa")
                nc.vector.tensor_scalar_mul(oa[:], agg_ps[:, :D2], d_loc[:, b:b + 1])
                ob = pq.tile([128, D2], F32, tag="ob")
                nc.vector.tensor_tensor(out=ob[:], in0=oa[:], in1=b2_t[:],
                                        op=mybir.AluOpType.add)
                nc.sync.dma_start(out=out_loc[b * 128:(b + 1) * 128, :], in_=ob[:])

    nc.compile()
    return nc


# ------------------------------------------------------------------- driver

def _prepare_inputs(h, W1, b1, W2, b2, src, dst):
    percore, meta = _preprocess(src, dst)
    hT = np.zeros((B, IN_D, NPAD), np.float32)
    hT[:, :, :N] = np.asarray(h, np.float32).transpose(0, 2, 1)
    b1r = np.tile(np.asarray(b1, np.float32), (128, B))
    b2r = np.tile(np.asarray(b2, np.float32), (128, B))
    cmax = max(max(meta["C_lo"][b] + meta["C_hi"][b] for b in range(PB)),
               max(meta["C_a"][b] + meta["C_b"][b] for b in range(PB)),
               max(meta["Sd"]))
    jr = np.tile(np.arange(128, dtype=np.float32), (128, cmax))
    idm = np.eye(128, dtype=np.float32)
    common = {
        "hT": hT, "w1": np.asarray(W1, np.float32), "w2": np.asarray(W2, np.float32),
        "b1r": b1r, "b2r": b2r, "jrep": jr, "ident": idm,
    }
    in_maps = [dict(common, **percore[c]) for c in range(NCORES)]
    return in_maps, meta


_BUILD_CACHE = {}


def _get_nc(meta):
    key = tuple(sorted((k, tuple(v) if isinstance(v, list) else v)
                       for k, v in meta.items()))
    if key not in _BUILD_CACHE:
        nc = _build(meta)
        nc.m = get_hw_module(nc.m)
        _BUILD_CACHE[key] = nc
    return _BUILD_CACHE[key]


def _assemble(results):
    full = np.concatenate([results[c]["out_loc"] for c in range(NCORES)], axis=0)
    out = full.reshape(NPAD, B, OUT_D).transpose(1, 0, 2)[:, :N, :]
    return np.ascontiguousarray(out, dtype=np.float32)


def kernel(h, W1, b1, W2, b2, src, dst):
    in_maps, meta = _prepare_inputs(h, W1, b1, W2, b2, src, dst)
    nc = _get_nc(meta)
    res = run_bass_kernel_spmd(nc, in_maps, core_ids=list(range(NCORES)))
    return _assemble(res.results)
